# revision 27
# baseline (speedup 1.0000x reference)
"""Beam-search top-k (mask pad + add beam scores + top-16 over beam*vocab) on 8 trn2 cores.

Sharding: batch dim (64 rows) split across 8 cores, 8 rows/core, no cross-core comm.

Per-core device pipeline (Bass/Tile, pure DVE selection -- no gpsimd topk):
  tile [128, 25136] f32, partition p = (t*8+b)*2 + h  (t=batch row, b=beam, h=half)
     h=0 holds vocab [0, 25136); h=1 holds vocab [25121, 50257)
  1. 16 chunked DMAs, each all-128-partitions (the (t,b) dims merge into one
     stride-50257 dim so src APs stay 3-D); per-chunk segmented reduce_max over
     groups of 16 -> M [128, 1571] pipelines with the DMAs. The pad-token
     (vocab 1) and h=0's copy of the 15-element overlap are fixed up directly
     in M after the reduces (3 narrow ops each) so no mask gates the pipeline.
  2. stage 1: per-partition top-16 groups of M via max8 / find_index8 /
     match_replace8 (HW resolves duplicate values to distinct positions in
     first-occurrence order, which matches jax.lax.top_k's lowest-index
     tie-break; beam score is constant per partition so selection is bias-free)
  3. prune: transpose the 16x16 group maxima of each token to [8, 256] and
     take the top-16 groups per token (same DVE chain). A winning element's
     group is in its partition's top-16 and in the token's top-16 groups, and
     first-occurrence order equals flat-index order, so this is exact.
  4. gather, 3 indirect DMAs (128 descriptors each): winner group col from a
     DRAM bounce of stage-1's index table; (rowbase, beam score) pairs from a
     host-built 2048x2 table; then the 16 raw elements of each winning group
     from x. Add the gathered score -> Gc2 [128, 16] (partition = token x rank)
  5. transpose to [8, 256] and take top-32 values + positions per token.
  6. host decodes positions through the index tables, drops raw pad-token
     entries, dedups h-overlap duplicates, sorts ties by flat index, takes 16.
"""

import sys

sys.path.insert(0, "/opt/trn_rl_repo")

import numpy as np

BSZ, BEAM, VOCAB, VK = 64, 8, 50257, 16
NCORES = 8
ROWS = BSZ // NCORES   # 8 tokens (batch rows) per core
F = 25136              # per-partition elems
CH0 = VOCAB - F        # 25121: h=1 partitions cover vocab [25121, 50257)
P = 128
GW = 16                # reduce group width
NG = F // GW           # 1571 groups per partition
LASTG = NG - 1         # group 1570 straddles the h=0 overlap
NEL = ROWS * BEAM * VOCAB  # 3216448 elements in the per-core shard
NEG = float("-inf")
NEGBIG = -3.0e38       # finite stand-in for -inf in match_replace imm (json-safe)

_CACHE = {}


def _build():
    import concourse.bacc as bacc
    import concourse.mybir as mybir
    from concourse.bass_types import AP
    from concourse.tile import TileContext
    from concourse.tile_rust import add_dep_helper

    ALU = mybir.AluOpType

    nc = bacc.Bacc("TRN2", target_bir_lowering=False, debug=False, num_swdge_queues=4)
    x = nc.dram_tensor("x", [ROWS, BEAM, VOCAB], mybir.dt.float32, kind="ExternalInput").ap()
    mcol = nc.dram_tensor("mcol", [P, 1], mybir.dt.float32, kind="ExternalInput").ap()
    scol = nc.dram_tensor("scol", [P, 1], mybir.dt.float32, kind="ExternalInput").ap()
    t256 = nc.dram_tensor("t256", [P, 1], mybir.dt.float32, kind="ExternalInput").ap()
    rbase = nc.dram_tensor("rbase", [P, 1], mybir.dt.float32, kind="ExternalInput").ap()
    e_s = nc.dram_tensor("e_s", [P * 16, 2], mybir.dt.uint32, kind="Internal").ap()

    o_i1 = nc.dram_tensor("o_i1", [P, 16], mybir.dt.uint32, kind="ExternalOutput").ap()
    o_ib2 = nc.dram_tensor("o_ib2", [ROWS, 16], mybir.dt.uint32, kind="ExternalOutput").ap()
    o_v = nc.dram_tensor("o_v", [ROWS, 32], mybir.dt.float32, kind="ExternalOutput").ap()
    o_i3b = nc.dram_tensor("o_i3b", [ROWS, 32], mybir.dt.uint32, kind="ExternalOutput").ap()

    with TileContext(nc) as tc:
        with tc.tile_pool(name="main", bufs=1) as pool:
            tile = pool.tile([P, F], mybir.dt.float32)
            M = pool.tile([P, NG], mybir.dt.float32)
            Mz = pool.tile([P, NG], mybir.dt.float32)
            mc = pool.tile([P, 1], mybir.dt.float32)
            sc = pool.tile([P, 1], mybir.dt.float32)
            A1b = pool.tile([P, 16], mybir.dt.float32)
            t2c = pool.tile([P, 1], mybir.dt.float32)
            rb = pool.tile([P, 1], mybir.dt.float32)
            r1 = pool.tile([P, 1], mybir.dt.float32)
            r2 = pool.tile([P, 1], mybir.dt.float32)
            r3 = pool.tile([P, 1], mybir.dt.float32)
            r4 = pool.tile([P, 1], mybir.dt.float32)
            A1 = pool.tile([P, 16], mybir.dt.float32)
            I1 = pool.tile([P, 16], mybir.dt.uint32)
            At = pool.tile([ROWS, 256], mybir.dt.float32)
            Atz = pool.tile([ROWS, 256], mybir.dt.float32)
            P0 = pool.tile([ROWS, 8], mybir.dt.float32)
            IB2 = pool.tile([ROWS, 16], mybir.dt.uint32)
            S2 = pool.tile([P, 1], mybir.dt.uint32)
            S2f = pool.tile([P, 1], mybir.dt.float32)
            Su2 = pool.tile([P, 1], mybir.dt.uint32)
            Sd2 = pool.tile([P, 1], mybir.dt.uint32)
            RS2 = pool.tile([P, 2], mybir.dt.uint32)
            Iff = pool.tile([P, 16], mybir.dt.float32)
            Ea = pool.tile([P, 16], mybir.dt.uint32)
            scb = pool.tile([P, 16], mybir.dt.float32)
            Gc2 = pool.tile([P, GW], mybir.dt.float32)
            Gt2 = pool.tile([ROWS, 256], mybir.dt.float32)
            Gz0 = pool.tile([ROWS, 256], mybir.dt.float32)
            Gz1 = pool.tile([ROWS, 256], mybir.dt.float32)
            AB = pool.tile([ROWS, 32], mybir.dt.float32)
            IB = pool.tile([ROWS, 32], mybir.dt.uint32)

            nc.sync.dma_start(out=mc[:, :], in_=mcol)
            nc.sync.dma_start(out=sc[:, :], in_=scol)
            nc.sync.dma_start(out=t2c[:, :], in_=t256)
            nc.sync.dma_start(out=rb[:, :], in_=rbase)

            # per-partition score broadcast [128,16], built early off-path
            nc.gpsimd.memset(scb[:, :], 0.0)
            nc.gpsimd.tensor_scalar_add(scb[:, :], scb[:, :], sc[:, 0:1])

            # chunked loads, all 128 partitions per DMA; per-chunk reduce
            chunks = [(i * 1568, 1568) for i in range(15)] + [(15 * 1568, 1616)]
            for o, ln in chunks:
                src = AP(
                    tensor=x.tensor, offset=o,
                    ap=[[VOCAB, ROWS * BEAM], [CH0, 2], [1, ln]],
                )
                nc.sync.dma_start(out=tile[:, o:o + ln], in_=src)
                t3 = tile[:, o:o + ln].rearrange("p (g w) -> p g w", w=GW)
                nc.vector.reduce_max(
                    out=M[:, o // GW:(o + ln) // GW], in_=t3, axis=mybir.AxisListType.X
                )

            # group-0 fixup: drop the pad token (vocab 1) from h=0 partitions.
            # M[:,0] = max(tile[:,0], tile[:,1] + mc, max(tile[:,2:16]))
            nc.vector.reduce_max(out=r1[:, :], in_=tile[:, 2:GW], axis=mybir.AxisListType.X)
            nc.vector.tensor_scalar_add(r2[:, :], tile[:, 1:2], mc[:, 0:1])
            nc.vector.tensor_tensor(out=r2[:, :], in0=r2[:, :], in1=r1[:, :], op=ALU.max)
            nc.vector.tensor_tensor(out=M[:, 0:1], in0=r2[:, :], in1=tile[:, 0:1], op=ALU.max)
            # last-group fixup: drop h=0's copy of the overlap [25121, 25136).
            # M[:,1570] = max(tile[:,25120], max(tile[:,25121:25136]) + mc)
            nc.vector.reduce_max(out=r3[:, :], in_=tile[:, CH0:F], axis=mybir.AxisListType.X)
            nc.vector.tensor_scalar_add(r4[:, :], r3[:, :], mc[:, 0:1])
            nc.vector.tensor_tensor(
                out=M[:, LASTG:LASTG + 1], in0=r4[:, :], in1=tile[:, GW * LASTG:GW * LASTG + 1],
                op=ALU.max,
            )

            # stage 1: per-partition top-16 groups
            nc.vector.max(out=A1[:, 0:8], in_=M[:, :])
            nc.vector.max_index(out=I1[:, 0:8], in_max=A1[:, 0:8], in_values=M[:, :])
            nc.vector.match_replace(
                out=Mz[:, :], in_to_replace=A1[:, 0:8], in_values=M[:, :],
                imm_value=NEGBIG,
            )
            nc.vector.max(out=A1[:, 8:16], in_=Mz[:, :])
            nc.vector.max_index(out=I1[:, 8:16], in_max=A1[:, 8:16], in_values=Mz[:, :])

            # all 256 candidate offsets up front: Ea = rowbase + col*GW (gpsimd,
            # so it overlaps DVE prune work); bounce (offset, score-bits) pairs
            # to DRAM for the winner gather
            nc.gpsimd.tensor_copy(out=Iff[:, :], in_=I1[:, :])
            nc.gpsimd.tensor_scalar(
                out=Iff[:, :], in0=Iff[:, :], scalar1=float(GW), scalar2=rb[:, 0:1],
                op0=ALU.mult, op1=ALU.add,
            )
            nc.gpsimd.tensor_copy(out=Ea[:, :], in_=Iff[:, :])
            e_sv = e_s.rearrange("(n c) o -> n (c o)", c=16)  # [128 rows, 32]
            w_es = nc.sync.dma_start(
                out=e_sv[:, :].rearrange("n (c two) -> n c two", two=2)[:, :, 0:1],
                in_=Ea[:, :],
            )
            w_es2 = nc.sync.dma_start(
                out=e_sv[:, :].rearrange("n (c two) -> n c two", two=2)[:, :, 1:2],
                in_=scb[:, :].bitcast(mybir.dt.uint32),
            )

            # prune: token-level top-16 groups from the 16x16 per-token maxima.
            # cross-partition means cross-beam, so bias by the beam score first.
            nc.vector.tensor_scalar_add(A1b[:, :], A1[:, :], sc[:, 0:1])
            nc.sync.dma_start(out=At[:, :], in_=A1b[:, :])
            nc.vector.max(out=P0[:, :], in_=At[:, :])
            nc.vector.max_index(out=IB2[:, 0:8], in_max=P0[:, :], in_values=At[:, :])
            nc.vector.match_replace(
                out=Atz[:, :], in_to_replace=P0[:, :], in_values=At[:, :],
                imm_value=NEGBIG,
            )
            nc.vector.max(out=P0[:, :], in_=Atz[:, :])
            nc.vector.max_index(out=IB2[:, 8:16], in_max=P0[:, :], in_values=Atz[:, :])

            # scatter token winners to partitions: S2[t*16+j] = IB2[t, j]
            nc.sync.dma_start(out=S2[:, :], in_=IB2[:, :])
            nc.sync.dma_start(out=o_ib2, in_=IB2[:, :])
            nc.sync.dma_start(out=o_i1, in_=I1[:, :])
            # Sd2 = 2*(t*256 + pos) for the 2-wide pair table (f32-exact; DVE
            # is idle here and wakes faster than gpsimd)
            nc.vector.tensor_copy(out=S2f[:, :], in_=S2[:, :])
            nc.vector.tensor_scalar(
                out=S2f[:, :], in0=S2f[:, :], scalar1=2.0, scalar2=t2c[:, 0:1],
                op0=ALU.mult, op1=ALU.add,
            )
            nc.vector.tensor_copy(out=Sd2[:, :], in_=S2f[:, :])

            import concourse.bass as bass

            def emit_indirect(out_ap, offs_ap, src_tensor, src_n, qi):
                g = nc.gpsimd
                src = AP(tensor=src_tensor, offset=0, ap=[[1, src_n], [1, 1]])
                in_ap = g.lower_ap_dma(src, for_indirect_dma=True)
                out_l = g.lower_ap_dma(out_ap, for_indirect_dma=True)
                off_l = g.lower_ap_dma(offs_ap)
                assert len(in_ap) == 1 and len(out_l) == 1 and len(off_l) == 1
                in_ap[0].dynamic_ap_info = mybir.DynamicAccessPatternInfo(
                    c=0,
                    actual_ap=out_ap.ap,
                    indirect_dim_max_index=src_n,
                    offset_expr=[
                        mybir.DynamicAccessPatternOffsetExpr(
                            coef=1,
                            aff_expr=mybir.DynamicAccessPatternOffsetExprAffExpr(
                                kind="IndirectArgId", arg_id=1,
                            ),
                        )
                    ],
                )
                in_ap.append(off_l[0])
                return g.add_instruction(
                    mybir.InstDMACopy(
                        name=nc.get_next_instruction_name(),
                        queue=f"qPoolDynamic{qi or ''}",
                        mode="Copy",
                        ins=in_ap,
                        outs=out_l,
                        oob_is_err=True,
                        cce_op=ALU.bypass,
                    )
                )

            g_rs = emit_indirect(RS2[:, 0:2], Sd2[:, 0:1], e_s.tensor, P * 16 * 2, 2)
            add_dep_helper(g_rs.ins, w_es.ins, reason="e_s DRAM bounce RAW")
            add_dep_helper(g_rs.ins, w_es2.ins, reason="e_s DRAM bounce RAW")
            emit_indirect(Gc2[:, 0:GW], RS2[:, 0:1], x.tensor, NEL, 3)
            nc.vector.tensor_scalar_add(
                Gc2[:, :], Gc2[:, :], RS2[:, 1:2].bitcast(mybir.dt.float32)
            )

            # transpose: token t's 16 winner-partitions -> one partition row
            nc.sync.dma_start(out=Gt2[:, :], in_=Gc2[:, :])

            # final: top-32 of each token's 256 candidates
            srcs = [Gt2, Gz0, Gz1, Gz0]
            for rd in range(4):
                s = srcs[rd]
                nc.vector.max(out=AB[:, rd * 8:rd * 8 + 8], in_=s[:, :])
                nc.vector.max_index(
                    out=IB[:, rd * 8:rd * 8 + 8], in_max=AB[:, rd * 8:rd * 8 + 8],
                    in_values=s[:, :],
                )
                if rd < 3:
                    nc.vector.match_replace(
                        out=srcs[rd + 1][:, :], in_to_replace=AB[:, rd * 8:rd * 8 + 8],
                        in_values=s[:, :], imm_value=NEGBIG,
                    )

            nc.sync.dma_start(out=o_v, in_=AB[:, :])
            nc.sync.dma_start(out=o_i3b, in_=IB[:, :])

    nc.compile()
    return nc


def _get_nc():
    if "nc" not in _CACHE:
        _CACHE["nc"] = _build()
    return _CACHE["nc"]


def _side_inputs(scores_shard: np.ndarray, step: int):
    mcol = np.zeros((P, 1), np.float32)
    scol = np.zeros((P, 1), np.float32)
    t256 = np.zeros((P, 1), np.float32)
    rbase = np.zeros((P, 1), np.float32)
    for t in range(ROWS):
        for b in range(BEAM):
            sv = (0.0 if b == 0 else NEG) if step == 0 else float(scores_shard[t, b])
            for h in range(2):
                p = t * 16 + b * 2 + h
                if h == 0:
                    mcol[p, 0] = NEG
                scol[p, 0] = sv
                q = b * 2 + h
                rowbase = float((t * BEAM + b) * VOCAB + h * CH0)
                rbase[p, 0] = rowbase
    for p in range(P):
        t256[p, 0] = float((p // 16) * 512)
    return mcol, scol, t256, rbase


def _decode(o_i1, o_ib2, o_v, o_i3b, step: int):
    vals = np.zeros((ROWS, VK), np.float32)
    vocab = np.zeros((ROWS, VK), np.int32)
    beams = np.zeros((ROWS, VK), np.int32)
    for t in range(ROWS):
        cand = []  # (val, beam, vocab)
        seen = set()
        vrow = o_v[t]
        exhausted = True  # capture covered everything down to the padding
        for s_ in range(32):
            val = vrow[s_]
            if val < -1e37 or not np.isfinite(val):
                break
            pos_b = int(o_i3b[t, s_])          # in [0, 256)
            j, e = divmod(pos_b, GW)
            q, r = divmod(int(o_ib2[t, j]), 16)
            col = int(o_i1[t * 16 + q, r])     # group col in [0, NG)
            b, h = divmod(q, 2)
            v = h * CH0 + col * GW + e
            if v == 1:
                continue  # pad token pulled in raw by the gather
            key = (b, v)
            if key in seen:
                continue  # h-overlap duplicate
            seen.add(key)
            cand.append((val, b, v))
        else:
            exhausted = False  # all 32 captured slots were live candidates
        assert len(cand) >= VK, f"only {len(cand)} unique candidates for row {t}"
        cand.sort(key=lambda c: (-c[0], c[1] * VOCAB + c[2]))
        # guard: if the 16th value ties with the last captured rank and the
        # capture wasn't exhaustive, a tie cluster might extend past the
        # top-32 window -- refuse rather than be silently wrong
        assert exhausted or cand[VK - 1][0] > vrow[31], (
            f"tie cluster may straddle the top-32 capture for row {t}"
        )
        for k in range(VK):
            vals[t, k] = cand[k][0]
            vocab[t, k] = cand[k][2]
            beams[t, k] = 0 if step == 0 else cand[k][1]
    return vals, vocab, beams


def _run(lprobs: np.ndarray, scores: np.ndarray, step: int, trace: bool = False):
    from concourse.bass_utils import run_bass_kernel_spmd

    nc = _get_nc()
    in_maps = []
    for c in range(NCORES):
        shard = np.ascontiguousarray(lprobs[c * ROWS:(c + 1) * ROWS])
        mcol, scol, t256, rbase = _side_inputs(scores[c * ROWS:(c + 1) * ROWS], step)
        in_maps.append({"x": shard, "mcol": mcol, "scol": scol, "t256": t256,
                        "rbase": rbase})
    res = run_bass_kernel_spmd(nc, in_maps, core_ids=list(range(NCORES)), trace=trace)
    return res


def kernel(lprobs, scores, step):
    lprobs = np.asarray(lprobs, dtype=np.float32)
    scores = np.asarray(scores, dtype=np.float32)
    step = int(step)

    res = _run(lprobs, scores, step)

    scores_buf = np.zeros((BSZ, VK), np.float32)
    indices_buf = np.zeros((BSZ, VK), np.int32)
    beams_buf = np.zeros((BSZ, VK), np.int32)
    for c in range(NCORES):
        o = res.results[c]
        v, vi, bi = _decode(o["o_i1"], o["o_ib2"], o["o_v"], o["o_i3b"], step)
        rows = slice(c * ROWS, (c + 1) * ROWS)
        scores_buf[rows] = v
        indices_buf[rows] = vi
        beams_buf[rows] = bi
    return scores_buf, indices_buf, beams_buf


# revision 28
# speedup vs baseline: 1.2873x; 1.2873x over previous
"""Beam-search top-k (mask pad + add beam scores + top-16 over beam*vocab) on 8 trn2 cores.

Sharding: batch dim (64 rows) split across 8 cores, 8 rows/core, no cross-core comm.

Per-core device pipeline (Bass/Tile, pure DVE selection -- no gpsimd topk):
  tile [128, 25136] f32, partition p = (t*8+b)*2 + h  (t=batch row, b=beam, h=half)
     h=0 holds vocab [0, 25136); h=1 holds vocab [25121, 50257)
  1. 16 chunked DMAs, each all-128-partitions (the (t,b) dims merge into one
     stride-50257 dim so src APs stay 3-D); per-chunk segmented reduce_max over
     groups of 16 -> M [128, 1571] pipelines with the DMAs. The pad-token
     (vocab 1) and h=0's copy of the 15-element overlap are fixed up directly
     in M after the reduces (3 narrow ops each) so no mask gates the pipeline.
  2. stage 1: per-partition top-16 groups of M via max8 / find_index8 /
     match_replace8 (HW resolves duplicate values to distinct positions in
     first-occurrence order, which matches jax.lax.top_k's lowest-index
     tie-break; beam score is constant per partition so selection is bias-free)
  3. prune: transpose the 16x16 group maxima of each token to [8, 256] and
     take the top-16 groups per token (same DVE chain). A winning element's
     group is in its partition's top-16 and in the token's top-16 groups, and
     first-occurrence order equals flat-index order, so this is exact.
  4. gather, 3 indirect DMAs (128 descriptors each): winner group col from a
     DRAM bounce of stage-1's index table; (rowbase, beam score) pairs from a
     host-built 2048x2 table; then the 16 raw elements of each winning group
     from x. Add the gathered score -> Gc2 [128, 16] (partition = token x rank)
  5. transpose to [8, 256] and take top-32 values + positions per token.
  6. host decodes positions through the index tables, drops raw pad-token
     entries, dedups h-overlap duplicates, sorts ties by flat index, takes 16.
"""

import sys

sys.path.insert(0, "/opt/trn_rl_repo")

import numpy as np

BSZ, BEAM, VOCAB, VK = 64, 8, 50257, 16
NCORES = 8
ROWS = BSZ // NCORES   # 8 tokens (batch rows) per core
F = 25136              # per-partition elems
CH0 = VOCAB - F        # 25121: h=1 partitions cover vocab [25121, 50257)
P = 128
GW = 16                # reduce group width
NG = F // GW           # 1571 groups per partition
LASTG = NG - 1         # group 1570 straddles the h=0 overlap
NEL = ROWS * BEAM * VOCAB  # 3216448 elements in the per-core shard
NEG = float("-inf")
NEGBIG = -3.0e38       # finite stand-in for -inf in match_replace imm (json-safe)

_CACHE = {}


def _build():
    import concourse.bacc as bacc
    import concourse.mybir as mybir
    from concourse.bass_types import AP
    from concourse.tile import TileContext
    from concourse.tile_rust import add_dep_helper

    ALU = mybir.AluOpType

    nc = bacc.Bacc("TRN2", target_bir_lowering=False, debug=False, num_swdge_queues=4)
    x = nc.dram_tensor("x", [ROWS, BEAM, VOCAB], mybir.dt.float32, kind="ExternalInput").ap()
    mcol = nc.dram_tensor("mcol", [P, 1], mybir.dt.float32, kind="ExternalInput").ap()
    scol = nc.dram_tensor("scol", [P, 1], mybir.dt.float32, kind="ExternalInput").ap()
    t256 = nc.dram_tensor("t256", [P, 1], mybir.dt.float32, kind="ExternalInput").ap()
    rbase = nc.dram_tensor("rbase", [P, 1], mybir.dt.float32, kind="ExternalInput").ap()
    e_s = nc.dram_tensor("e_s", [P * 16, 1], mybir.dt.uint32, kind="Internal").ap()
    s_tab = nc.dram_tensor("s_tab", [P * 16, 1], mybir.dt.float32, kind="ExternalInput").ap()

    o_i1 = nc.dram_tensor("o_i1", [P, 16], mybir.dt.uint32, kind="ExternalOutput").ap()
    o_ib2 = nc.dram_tensor("o_ib2", [ROWS, 16], mybir.dt.uint32, kind="ExternalOutput").ap()
    o_v = nc.dram_tensor("o_v", [ROWS, 32], mybir.dt.float32, kind="ExternalOutput").ap()
    o_i3b = nc.dram_tensor("o_i3b", [ROWS, 32], mybir.dt.uint32, kind="ExternalOutput").ap()

    with TileContext(nc) as tc:
        with tc.tile_pool(name="main", bufs=1) as pool:
            tile = pool.tile([P, F], mybir.dt.float32)
            M = pool.tile([P, NG], mybir.dt.float32)
            Mz = pool.tile([P, NG], mybir.dt.float32)
            mc = pool.tile([P, 1], mybir.dt.float32)
            sc = pool.tile([P, 1], mybir.dt.float32)
            A1b = pool.tile([P, 16], mybir.dt.float32)
            t2c = pool.tile([P, 1], mybir.dt.float32)
            rb = pool.tile([P, 1], mybir.dt.float32)
            r1 = pool.tile([P, 1], mybir.dt.float32)
            r2 = pool.tile([P, 1], mybir.dt.float32)
            r3 = pool.tile([P, 1], mybir.dt.float32)
            r4 = pool.tile([P, 1], mybir.dt.float32)
            A1 = pool.tile([P, 16], mybir.dt.float32)
            I1 = pool.tile([P, 16], mybir.dt.uint32)
            At = pool.tile([ROWS, 256], mybir.dt.float32)
            Atz = pool.tile([ROWS, 256], mybir.dt.float32)
            P0 = pool.tile([ROWS, 8], mybir.dt.float32)
            IB2 = pool.tile([ROWS, 16], mybir.dt.uint32)
            S2 = pool.tile([P, 1], mybir.dt.uint32)
            S2f = pool.tile([P, 1], mybir.dt.float32)
            Su2 = pool.tile([P, 1], mybir.dt.uint32)
            Eu = pool.tile([P, 1], mybir.dt.uint32)
            Sgt = pool.tile([P, 1], mybir.dt.float32)
            Iff = pool.tile([P, 16], mybir.dt.float32)
            Ea = pool.tile([P, 16], mybir.dt.uint32)
            Gc2 = pool.tile([P, GW], mybir.dt.float32)
            Gt2 = pool.tile([ROWS, 256], mybir.dt.float32)
            Gz0 = pool.tile([ROWS, 256], mybir.dt.float32)
            Gz1 = pool.tile([ROWS, 256], mybir.dt.float32)
            AB = pool.tile([ROWS, 32], mybir.dt.float32)
            IB = pool.tile([ROWS, 32], mybir.dt.uint32)

            nc.sync.dma_start(out=mc[:, :], in_=mcol)
            nc.sync.dma_start(out=sc[:, :], in_=scol)
            nc.sync.dma_start(out=t2c[:, :], in_=t256)
            nc.sync.dma_start(out=rb[:, :], in_=rbase)


            # chunked loads, all 128 partitions per DMA; per-chunk reduce
            chunks = [(i * 1568, 1568) for i in range(15)] + [(15 * 1568, 1616)]
            for o, ln in chunks:
                src = AP(
                    tensor=x.tensor, offset=o,
                    ap=[[VOCAB, ROWS * BEAM], [CH0, 2], [1, ln]],
                )
                nc.sync.dma_start(out=tile[:, o:o + ln], in_=src)
                t3 = tile[:, o:o + ln].rearrange("p (g w) -> p g w", w=GW)
                nc.vector.reduce_max(
                    out=M[:, o // GW:(o + ln) // GW], in_=t3, axis=mybir.AxisListType.X
                )

            # group-0 fixup: drop the pad token (vocab 1) from h=0 partitions.
            # M[:,0] = max(tile[:,0], tile[:,1] + mc, max(tile[:,2:16]))
            nc.vector.reduce_max(out=r1[:, :], in_=tile[:, 2:GW], axis=mybir.AxisListType.X)
            nc.vector.tensor_scalar_add(r2[:, :], tile[:, 1:2], mc[:, 0:1])
            nc.vector.tensor_tensor(out=r2[:, :], in0=r2[:, :], in1=r1[:, :], op=ALU.max)
            nc.vector.tensor_tensor(out=M[:, 0:1], in0=r2[:, :], in1=tile[:, 0:1], op=ALU.max)
            # last-group fixup: drop h=0's copy of the overlap [25121, 25136).
            # M[:,1570] = max(tile[:,25120], max(tile[:,25121:25136]) + mc)
            nc.vector.reduce_max(out=r3[:, :], in_=tile[:, CH0:F], axis=mybir.AxisListType.X)
            nc.vector.tensor_scalar_add(r4[:, :], r3[:, :], mc[:, 0:1])
            nc.vector.tensor_tensor(
                out=M[:, LASTG:LASTG + 1], in0=r4[:, :], in1=tile[:, GW * LASTG:GW * LASTG + 1],
                op=ALU.max,
            )

            # stage 1: per-partition top-16 groups
            nc.vector.max(out=A1[:, 0:8], in_=M[:, :])
            nc.vector.max_index(out=I1[:, 0:8], in_max=A1[:, 0:8], in_values=M[:, :])
            nc.vector.match_replace(
                out=Mz[:, :], in_to_replace=A1[:, 0:8], in_values=M[:, :],
                imm_value=NEGBIG,
            )
            nc.vector.max(out=A1[:, 8:16], in_=Mz[:, :])
            nc.vector.max_index(out=I1[:, 8:16], in_max=A1[:, 8:16], in_values=Mz[:, :])

            # prune: token-level top-16 groups from the 16x16 per-token maxima.
            # cross-partition means cross-beam, so bias by the beam score first.
            nc.vector.tensor_scalar_add(A1b[:, :], A1[:, :], sc[:, 0:1])
            nc.sync.dma_start(out=At[:, :], in_=A1b[:, :])

            # all 256 candidate offsets up front: Ea = rowbase + col*GW (gpsimd,
            # so it overlaps DVE prune work); bounce to DRAM for the winner gather
            nc.gpsimd.tensor_copy(out=Iff[:, :], in_=I1[:, :])
            nc.gpsimd.tensor_scalar(
                out=Iff[:, :], in0=Iff[:, :], scalar1=float(GW), scalar2=rb[:, 0:1],
                op0=ALU.mult, op1=ALU.add,
            )
            nc.gpsimd.tensor_copy(out=Ea[:, :], in_=Iff[:, :])
            w_es = nc.sync.dma_start(out=e_s, in_=Ea[:, :])
            nc.vector.max(out=P0[:, :], in_=At[:, :])
            nc.vector.max_index(out=IB2[:, 0:8], in_max=P0[:, :], in_values=At[:, :])
            nc.vector.match_replace(
                out=Atz[:, :], in_to_replace=P0[:, :], in_values=At[:, :],
                imm_value=NEGBIG,
            )
            nc.vector.max(out=P0[:, :], in_=Atz[:, :])
            nc.vector.max_index(out=IB2[:, 8:16], in_max=P0[:, :], in_values=Atz[:, :])

            # scatter token winners to partitions: S2[t*16+j] = IB2[t, j]
            nc.sync.dma_start(out=S2[:, :], in_=IB2[:, :])
            nc.sync.dma_start(out=o_ib2, in_=IB2[:, :])
            nc.sync.dma_start(out=o_i1, in_=I1[:, :])
            # Su2 = t*256 + pos (f32-exact; DVE is idle here and wakes faster)
            nc.vector.tensor_copy(out=S2f[:, :], in_=S2[:, :])
            nc.vector.tensor_tensor(out=S2f[:, :], in0=S2f[:, :], in1=t2c[:, 0:1], op=ALU.add)
            nc.vector.tensor_copy(out=Su2[:, :], in_=S2f[:, :])

            import concourse.bass as bass

            def emit_indirect(out_ap, offs_ap, src_tensor, src_n, qi):
                g = nc.gpsimd
                src = AP(tensor=src_tensor, offset=0, ap=[[1, src_n], [1, 1]])
                in_ap = g.lower_ap_dma(src, for_indirect_dma=True)
                out_l = g.lower_ap_dma(out_ap, for_indirect_dma=True)
                off_l = g.lower_ap_dma(offs_ap)
                assert len(in_ap) == 1 and len(out_l) == 1 and len(off_l) == 1
                in_ap[0].dynamic_ap_info = mybir.DynamicAccessPatternInfo(
                    c=0,
                    actual_ap=out_ap.ap,
                    indirect_dim_max_index=src_n,
                    offset_expr=[
                        mybir.DynamicAccessPatternOffsetExpr(
                            coef=1,
                            aff_expr=mybir.DynamicAccessPatternOffsetExprAffExpr(
                                kind="IndirectArgId", arg_id=1,
                            ),
                        )
                    ],
                )
                in_ap.append(off_l[0])
                return g.add_instruction(
                    mybir.InstDMACopy(
                        name=nc.get_next_instruction_name(),
                        queue=f"qPoolDynamic{qi or ''}",
                        mode="Copy",
                        ins=in_ap,
                        outs=out_l,
                        oob_is_err=True,
                        cce_op=ALU.bypass,
                    )
                )

            g_eu = emit_indirect(Eu[:, 0:1], Su2[:, 0:1], e_s.tensor, P * 16, 1)
            add_dep_helper(g_eu.ins, w_es.ins, reason="e_s DRAM bounce RAW")
            emit_indirect(Sgt[:, 0:1], Su2[:, 0:1], s_tab.tensor, P * 16, 2)
            emit_indirect(Gc2[:, 0:GW], Eu[:, 0:1], x.tensor, NEL, 3)
            nc.vector.tensor_scalar_add(Gc2[:, :], Gc2[:, :], Sgt[:, 0:1])

            # transpose: token t's 16 winner-partitions -> one partition row
            nc.sync.dma_start(out=Gt2[:, :], in_=Gc2[:, :])

            # final: top-32 of each token's 256 candidates
            srcs = [Gt2, Gz0, Gz1, Gz0]
            for rd in range(4):
                s = srcs[rd]
                nc.vector.max(out=AB[:, rd * 8:rd * 8 + 8], in_=s[:, :])
                nc.vector.max_index(
                    out=IB[:, rd * 8:rd * 8 + 8], in_max=AB[:, rd * 8:rd * 8 + 8],
                    in_values=s[:, :],
                )
                if rd < 3:
                    nc.vector.match_replace(
                        out=srcs[rd + 1][:, :], in_to_replace=AB[:, rd * 8:rd * 8 + 8],
                        in_values=s[:, :], imm_value=NEGBIG,
                    )

            nc.sync.dma_start(out=o_v, in_=AB[:, :])
            nc.sync.dma_start(out=o_i3b, in_=IB[:, :])

    nc.compile()
    return nc


def _get_nc():
    if "nc" not in _CACHE:
        _CACHE["nc"] = _build()
    return _CACHE["nc"]


def _side_inputs(scores_shard: np.ndarray, step: int):
    mcol = np.zeros((P, 1), np.float32)
    scol = np.zeros((P, 1), np.float32)
    t256 = np.zeros((P, 1), np.float32)
    rbase = np.zeros((P, 1), np.float32)
    s_tab = np.zeros((P * 16, 1), np.float32)
    for t in range(ROWS):
        for b in range(BEAM):
            sv = (0.0 if b == 0 else NEG) if step == 0 else float(scores_shard[t, b])
            for h in range(2):
                p = t * 16 + b * 2 + h
                if h == 0:
                    mcol[p, 0] = NEG
                scol[p, 0] = sv
                q = b * 2 + h
                rowbase = float((t * BEAM + b) * VOCAB + h * CH0)
                rbase[p, 0] = rowbase
                for r in range(16):
                    s_tab[t * 256 + q * 16 + r, 0] = sv
    for p in range(P):
        t256[p, 0] = float((p // 16) * 256)
    return mcol, scol, t256, rbase, s_tab


def _decode(o_i1, o_ib2, o_v, o_i3b, step: int):
    vals = np.zeros((ROWS, VK), np.float32)
    vocab = np.zeros((ROWS, VK), np.int32)
    beams = np.zeros((ROWS, VK), np.int32)
    for t in range(ROWS):
        cand = []  # (val, beam, vocab)
        seen = set()
        vrow = o_v[t]
        exhausted = True  # capture covered everything down to the padding
        for s_ in range(32):
            val = vrow[s_]
            if val < -1e37 or not np.isfinite(val):
                break
            pos_b = int(o_i3b[t, s_])          # in [0, 256)
            j, e = divmod(pos_b, GW)
            q, r = divmod(int(o_ib2[t, j]), 16)
            col = int(o_i1[t * 16 + q, r])     # group col in [0, NG)
            b, h = divmod(q, 2)
            v = h * CH0 + col * GW + e
            if v == 1:
                continue  # pad token pulled in raw by the gather
            key = (b, v)
            if key in seen:
                continue  # h-overlap duplicate
            seen.add(key)
            cand.append((val, b, v))
        else:
            exhausted = False  # all 32 captured slots were live candidates
        assert len(cand) >= VK, f"only {len(cand)} unique candidates for row {t}"
        cand.sort(key=lambda c: (-c[0], c[1] * VOCAB + c[2]))
        # guard: if the 16th value ties with the last captured rank and the
        # capture wasn't exhaustive, a tie cluster might extend past the
        # top-32 window -- refuse rather than be silently wrong
        assert exhausted or cand[VK - 1][0] > vrow[31], (
            f"tie cluster may straddle the top-32 capture for row {t}"
        )
        for k in range(VK):
            vals[t, k] = cand[k][0]
            vocab[t, k] = cand[k][2]
            beams[t, k] = 0 if step == 0 else cand[k][1]
    return vals, vocab, beams


def _run(lprobs: np.ndarray, scores: np.ndarray, step: int, trace: bool = False):
    from concourse.bass_utils import run_bass_kernel_spmd

    nc = _get_nc()
    in_maps = []
    for c in range(NCORES):
        shard = np.ascontiguousarray(lprobs[c * ROWS:(c + 1) * ROWS])
        mcol, scol, t256, rbase, s_tab = _side_inputs(scores[c * ROWS:(c + 1) * ROWS], step)
        in_maps.append({"x": shard, "mcol": mcol, "scol": scol, "t256": t256,
                        "rbase": rbase, "s_tab": s_tab})
    res = run_bass_kernel_spmd(nc, in_maps, core_ids=list(range(NCORES)), trace=trace)
    return res


def kernel(lprobs, scores, step):
    lprobs = np.asarray(lprobs, dtype=np.float32)
    scores = np.asarray(scores, dtype=np.float32)
    step = int(step)

    res = _run(lprobs, scores, step)

    scores_buf = np.zeros((BSZ, VK), np.float32)
    indices_buf = np.zeros((BSZ, VK), np.int32)
    beams_buf = np.zeros((BSZ, VK), np.int32)
    for c in range(NCORES):
        o = res.results[c]
        v, vi, bi = _decode(o["o_i1"], o["o_ib2"], o["o_v"], o["o_i3b"], step)
        rows = slice(c * ROWS, (c + 1) * ROWS)
        scores_buf[rows] = v
        indices_buf[rows] = vi
        beams_buf[rows] = bi
    return scores_buf, indices_buf, beams_buf
